# revision 21
# baseline (speedup 1.0000x reference)
"""Min-norm solver (MGDA) for Trainium2, sharded across 8 NeuronCores.

Strategy:
  - vecs is [32, 2097152] f32 (256 MB).  The only memory-heavy step is the
    Gram matrix G = vecs @ vecs.T ([32, 32]).  We shard the d dimension
    across 8 cores and compute partial Grams on-device.
  - On-device layout: the host pre-transposes each core's shard into
    X[p, n*32 + j] = vecs[j, n*128 + p]  (p: 0..127 partition, n: d-chunk,
    j: task), so the TensorEngine can contract over the partition dim with
    fully-contiguous APs.  Four d-chunks are packed into one [128, 128]
    operand; one LDW + matmul per group accumulates all 4 chunks' partial
    Grams into the 4 diagonal [32,32] blocks of a [128,128] PSUM tile.
  - Precision/bandwidth trade: fp32 matmul on TRN2 costs 4 cycles/row
    (PE-bound above the DMA roofline), so vecs is cast to fp16 (11-bit
    mantissa, |v| <= ~6 so no range issues).  The resulting Gram error
    (~3.8 absolute on a 2.1e6 diagonal) is the same magnitude as the f32
    reference's own accumulation error; end-to-end solution error vs the
    f32 reference is ~3.7e-6, about 2x the fp32 cross-platform envelope.
    This halves DMA traffic to 16 MB/core and the PE runs at 1 cycle/row.
    (Fallback encodings kept in the code: hi bf16 + fp8e4m3 lo scaled by
    512, G = H^T H + (H^T L' + L'^T H)/512, gives 1.2e-6 at 24 MB/core.)
  - The tiny 250-iteration solver runs on the host in float32 numpy,
    faithfully mirroring the reference ops.
"""

import numpy as np
import ml_dtypes

N_TASKS = 32
D = 2097152
N_CORES = 8
D_LOC = D // N_CORES          # 262144 d-values per core
N_CHUNK = D_LOC // 128        # 2048 chunks of 128 d-values
TILE_FREE = 2048              # SBUF-tile columns (16 groups of 128)
LO_SCALE = np.float32(512.0)  # keeps lo inside fp8e4m3's normal range

MAX_ITER = 250
STOP_CRIT = np.float32(1e-6)
EPS = np.float32(1e-8)

LO_MODE = "none"              # "fp8" | "bf16" | "none"
HI_DTYPE = "fp16"             # "fp16" | "bf16"

_PROGRAMS = {}


def _np_lo_dtype(lo_mode):
    return {"fp8": ml_dtypes.float8_e4m3, "bf16": ml_dtypes.bfloat16}[lo_mode]


def _build_program(tile_free=TILE_FREE, bufs=6, dma="sync", lo_mode=LO_MODE,
                   tail_split=2, copy_eng="vector", hi_dtype=HI_DTYPE):
    import concourse.bass as bass
    import concourse.mybir as mybir
    import concourse.tile as tile
    from concourse import bacc

    total_free = N_CHUNK * 32
    # column extents per SBUF tile; optionally split the last tile into
    # small pieces so the final DMA->matmul tail is short
    edges = list(range(0, total_free, tile_free))
    widths = [tile_free] * len(edges)
    if tail_split:
        e0 = edges.pop()
        widths.pop()
        step = tile_free // tail_split
        for k in range(tail_split):
            edges.append(e0 + k * step)
            widths.append(step)
    have_lo = lo_mode != "none"
    lo_dt = {"fp8": mybir.dt.float8e4, "bf16": mybir.dt.bfloat16,
             "none": None}[lo_mode]
    hi_dt = {"fp16": mybir.dt.float16, "bf16": mybir.dt.bfloat16}[hi_dtype]

    out_w = 256 if have_lo else 128

    nc = bacc.Bacc("TRN2", target_bir_lowering=False, debug=False,
                   num_devices=N_CORES)
    xh = nc.dram_tensor("xh", [128, N_CHUNK * 32], hi_dt,
                        kind="ExternalInput").ap()
    if have_lo:
        xl = nc.dram_tensor("xl", [128, N_CHUNK * 32], lo_dt,
                            kind="ExternalInput").ap()
    out_ab = nc.dram_tensor("out_ab", [128, out_w], mybir.dt.float32,
                            kind="ExternalOutput").ap()

    with tile.TileContext(nc) as tc:
        with (
            tc.tile_pool(name="hi", bufs=bufs) as hi_pool,
            tc.tile_pool(name="lo", bufs=bufs) as lo_pool,
            tc.tile_pool(name="psum", bufs=1, space="PSUM") as psum_pool,
            tc.tile_pool(name="outs", bufs=1) as out_pool,
        ):
            dma_eng = getattr(nc, dma)
            # separate PSUM tiles (one bank each): interleaved accumulation
            # groups sharing a bank corrupt each other's start_tensor_calc
            p_a = psum_pool.tile([128, 128], mybir.dt.float32, name="p_a")
            p_b = (psum_pool.tile([128, 128], mybir.dt.float32, name="p_b")
                   if have_lo else None)
            for t, (e, w) in enumerate(zip(edges, widths)):
                ht = hi_pool.tile([128, w], hi_dt, tag="ht")
                dma_eng.dma_start(ht[:], xh[:, e:e + w])
                if have_lo:
                    lt = lo_pool.tile([128, w], lo_dt, tag="lt")
                    dma_eng.dma_start(lt[:], xl[:, e:e + w])
                for g in range(w // 128):
                    sl = bass.ts(g, 128)
                    first = t == 0 and g == 0
                    last = t == len(edges) - 1 and g == w // 128 - 1
                    nc.tensor.matmul(p_a[:], ht[:, sl], ht[:, sl],
                                     start=first, stop=last)
                    if have_lo:
                        nc.tensor.matmul(p_b[:], ht[:, sl], lt[:, sl],
                                         start=first, stop=last)
            o_ab = out_pool.tile([128, out_w], mybir.dt.float32)
            if copy_eng == "scalar":
                nc.scalar.copy(o_ab[:, 0:128], p_a[:])
                if have_lo:
                    nc.scalar.copy(o_ab[:, 128:256], p_b[:])
            else:
                nc.vector.tensor_copy(o_ab[:, 0:128], p_a[:])
                if have_lo:
                    nc.vector.tensor_copy(o_ab[:, 128:256], p_b[:])
            nc.sync.dma_start(out_ab, o_ab[:])
    nc.compile()
    return nc


def _get_program(**kw):
    key = tuple(sorted(kw.items()))
    if key not in _PROGRAMS:
        _PROGRAMS[key] = _build_program(**kw)
    return _PROGRAMS[key]


def _prep_inputs(vecs, lo_mode=LO_MODE, hi_dtype=HI_DTYPE):
    """[32, D] f32 -> per-core hi (fp16/bf16) / lo arrays in PE layout.

    X[c, p, n*32 + j] = vecs[j, c*D_LOC + n*128 + p]
    """
    x = np.asarray(vecs, dtype=np.float32)
    x = x.reshape(N_TASKS, N_CORES, N_CHUNK, 128)      # [j, c, n, p]
    x = np.ascontiguousarray(x.transpose(1, 3, 2, 0))  # [c, p, n, j]
    x = x.reshape(N_CORES, 128, N_CHUNK * 32)
    hi_np = {"fp16": np.float16, "bf16": ml_dtypes.bfloat16}[hi_dtype]
    hi = x.astype(hi_np)
    if lo_mode == "none":
        return hi, None
    lo = x - hi.astype(np.float32)
    if lo_mode == "fp8":
        lo *= LO_SCALE
    lo = lo.astype(_np_lo_dtype(lo_mode))
    return hi, lo


def run_device(vecs, lo_mode=LO_MODE, hi_dtype=HI_DTYPE, **prog_kw):
    """Run the sharded Gram computation; returns (G [32,32] f32, results)."""
    from concourse.bass_utils import run_bass_kernel_spmd

    hi, lo = _prep_inputs(vecs, lo_mode, hi_dtype)
    if lo is None:
        in_maps = [{"xh": hi[c]} for c in range(N_CORES)]
    else:
        in_maps = [{"xh": hi[c], "xl": lo[c]} for c in range(N_CORES)]
    res = run_bass_kernel_spmd(
        _get_program(lo_mode=lo_mode, hi_dtype=hi_dtype, **prog_kw),
        in_maps, list(range(N_CORES)))
    lo_rescale = 1.0 / float(LO_SCALE) if lo_mode == "fp8" else 1.0
    g_acc = np.zeros((N_TASKS, N_TASKS), dtype=np.float64)
    for c in range(N_CORES):
        ab = res.results[c]["out_ab"].astype(np.float64)
        a = ab[:, 0:128]
        b = ab[:, 128:256] * lo_rescale if lo is not None else None
        for s in range(4):
            blk = slice(32 * s, 32 * (s + 1))
            g_acc += a[blk, blk]
            if b is not None:
                g_acc += b[blk, blk] + b[blk, blk].T
    return g_acc.astype(np.float32), res


# ---------------------------------------------------------------------------
# Host-side solver: faithful float32 numpy port of the reference iteration.
# ---------------------------------------------------------------------------

def _line_solver(v11, v12, v22):
    g = (v22 - v12) / (v11 + v22 - np.float32(2.0) * v12 + EPS)
    c = v22 + g * (v12 - v22)
    gamma = np.where(v12 >= v22, np.float32(0.0), g)
    gamma = np.where(v12 >= v11, np.float32(1.0), gamma)
    cost = np.where(v12 >= v22, v22, c)
    cost = np.where(v12 >= v11, v11, cost)
    return gamma.astype(np.float32), cost.astype(np.float32)


def _planar_init(G, n):
    iu, ju = np.triu_indices(n, 1)
    vivj = G[iu, ju]
    vivi = G[iu, iu]
    vjvj = G[ju, ju]
    gamma, cost = _line_solver(vivi, vivj, vjvj)
    off = int(np.argmin(cost))
    sol = np.zeros(n, dtype=G.dtype)
    sol[iu[off]] = gamma[off]
    sol[ju[off]] = np.float32(1.0) - gamma[off]
    return sol


def _proj_simplex(gamma, i_grid):
    s = np.sort(gamma)[::-1]  # descending
    tmp_max = (np.cumsum(s, dtype=np.float32) - np.float32(1.0)) / i_grid
    cond = tmp_max[:-1] > s[1:]
    first = int(np.argmax(cond))  # first True (0 if none)
    tmax = tmp_max[:-1][first] if bool(np.any(cond)) else tmp_max[-1]
    return np.maximum(gamma - tmax, np.float32(0.0)).astype(np.float32)


def _next_point(cur, grad, n_f, i_grid):
    proj = (grad - np.sum(grad) / n_f).astype(np.float32)
    neg = proj < 0
    pos = proj > 0
    inf = np.float32(np.inf)
    tm1 = np.where(neg, -cur / np.where(neg, proj, np.float32(1.0)), inf)
    tm2 = np.where(pos, (np.float32(1.0) - cur) / np.where(pos, proj, np.float32(1.0)), inf)
    thr = np.float32(1e-7)
    m1 = np.min(np.where(tm1 > thr, tm1, inf))
    t = m1 if np.isfinite(m1) else np.float32(1.0)
    m2 = np.min(np.where(tm2 > thr, tm2, inf))
    t = np.minimum(t, m2).astype(np.float32)
    nxt = (proj * t + cur).astype(np.float32)
    return _proj_simplex(nxt, i_grid)


def solve(G):
    n = G.shape[0]
    sol = _planar_init(G, n)
    i_grid = (np.arange(n, dtype=G.dtype) + np.float32(1.0)).astype(G.dtype)
    n_f = np.float32(n)
    for _ in range(MAX_ITER):
        grad_dir = (-(G @ sol)).astype(np.float32)
        newp = _next_point(sol, grad_dir, n_f, i_grid)
        gs = G @ sol
        gn = G @ newp
        v11 = np.float32(sol @ gs)
        v12 = np.float32(sol @ gn)
        v22 = np.float32(newp @ gn)
        gamma, _ = _line_solver(v11, v12, v22)
        new_sol = (gamma * sol + (np.float32(1.0) - gamma) * newp).astype(np.float32)
        if np.sum(np.abs(new_sol - sol)) < STOP_CRIT:
            break  # reference freezes the OLD sol once change < stop_crit
        sol = new_sol
    return sol.astype(np.float32)


def kernel(vecs):
    G, _ = run_device(vecs)
    return solve(G)
